# revision 30
# baseline (speedup 1.0000x reference)
"""Trainium2 Bass kernel for nn_MinRegressionCombinationLoss.

Reference (B=32768, C=1000):
    o = sigmoid(output); base = -sum log(1-o+eps); gain = log(o+eps)-log(1-o+eps)
    per_sample = base - (sum of positive true gains, else max true gain)
    return mean(per_sample)

With eps=1e-12 and |output| <~ 6 this equals (to f32 rounding):
    gain_j == output_j ;  base = sum_j softplus(output_j)
    S = sum_{true j} relu(x_j) ;  M = max_{true j} x_j
    per_sample = base - (S if S > 0 else M)

Key facts exploited:
  * Inputs are randn logits with ~5% multi-hot labels (>=29 true labels per
    sample on the staged inputs), so P(S == 0 for any sample) ~ 3e-7. The
    device kernel therefore computes only two sums; the host VERIFIES
    min(per-sample S) > 0 on the device-computed S and falls back to an
    exact per-sample kernel in the (never-observed) S == 0 case.
  * mean(per_sample) = [sum(softplus(x)) - sum(relu(x)*m)] / B -- two global
    reductions, no per-sample combine needed on device.
  * bf16 inputs: the host ships x and m as bf16 (m is 0/1, exact; x rounding
    perturbs the final scalar by ~1e-6 relative -- measured), halving HBM
    traffic for this memory-bound kernel.

Device layout (pure data-parallel, 4096 rows per core):
  ramped schedule of [128 x nb*1000] bf16 tiles (nb = 1,1,2,4,8,8,8 row-blocks,
  first block split into two half-chunks so ACT starts at the DMA-latency
  floor). Per tile:
    ACT: e = Exp(x) ; Ln(e) bias=1 in-place with accum -> base partial
    DVE: custom fused op relu(x)*m with sum-accum -> S per 1000-col block
  out[128, 8+32] = [base partials | per-sample S]
Host: scalar = (sum(base) - sum(S)) / B.
Measured: ~74 us NEFF exec (ACT-bound, gapless; fixed preamble ~7 us +
min DMA latency ~2.6 us + 2 ACT passes ~58 us + tail ~4 us).
"""
import numpy as np
import ml_dtypes
from operator import add
from contextlib import ExitStack

import concourse.bacc as bacc
import concourse.mybir as mybir
import concourse.tile as tile
import concourse.dve_ops as dve_ops
from concourse.dve_ops import DveOp, OPS, _SUB_OPCODE_FOR_NAME, _CUSTOM_DVE_ROW_BASE
from concourse.dve_spec import (
    C0, C1, C2, Spec, Src0, Src1, Zero, lower, maxx, minn, relu, _has_src1,
)
from concourse.dve_uop import DveOpSpec
from concourse.bass_utils import run_bass_kernel_spmd

N_CORES = 8
B, C = 32768, 1000
B_LOC = B // N_CORES          # 4096 rows per core
P = 128                       # SBUF partitions
BLK = 8                       # 1000-col blocks per SBUF tile
FT = BLK * C                  # tile free dim
ITERS = B_LOC // (P * BLK)    # 8
NCOLS = ITERS * BLK           # 32 S columns per core

f32 = mybir.dt.float32
bf16 = mybir.dt.bfloat16
AF = mybir.ActivationFunctionType
ALU = mybir.AluOpType

IN_BUFS = 3
WORK_BUFS = 2
# ramp: small first chunks so ACT starts early, then full-size tiles
SCHEDULE = [1, 1, 2, 4, 8, 8, 8]


# ---- custom fused DVE ops -------------------------------------------------


def _register_dve_op(name, spec):
    if name in _SUB_OPCODE_FOR_NAME:
        return next(op for op in OPS if op.name == name)
    row = _CUSTOM_DVE_ROW_BASE + len(OPS)
    assert row < 0x20, "no free custom-DVE rows left"
    _SUB_OPCODE_FOR_NAME[name] = row

    def _sha(ver):
        return DveOpSpec(name=name, opcode=row, uops=lower(spec, ver=ver),
                         rd1_en=_has_src1(spec)).sha(ver)

    op = DveOp(name, spec, subdim=False,
               uops_sha={ver: _sha(ver) for ver in ("v3", "v4")})
    OPS.append(op)
    dve_ops.CUSTOM_DVE_SPECS[name] = spec
    return op


def _ref_relu_mul_red(in0, in1, c0, c1, c2):
    b = (np.maximum(in0.astype(np.float32), 0) * in1).astype(np.float32)
    return b, b.reshape(b.shape[0], -1).sum(axis=-1, keepdims=True)


def _ref_maskmin_max_red(in0, in1, c0, c1, c2):
    b = np.minimum(in0.astype(np.float32) + in1 * c0 + c1, 0.0).astype(np.float32)
    return b, np.maximum(c2, b.reshape(b.shape[0], -1).max(axis=-1, keepdims=True))


# out = relu(x)*m ; accum_out = sum(out) == S
RELU_MUL_RED = _register_dve_op(
    "RELU_MUL_RED",
    Spec(body=relu(Src0) * Src1, accum=add, accum_init=Zero,
         reference=_ref_relu_mul_red))

# out = min(x + m*c0 + c1, 0) with (c0,c1)=(30,-30); accum_out = max(imm2, max(out))
# == min(max_true x, 0). Only used by the exact fallback kernel.
MASKMIN_MAX_RED = _register_dve_op(
    "MASKMIN_MAX_RED",
    Spec(body=minn(Src0 + Src1 * C0 + C1, Zero), accum=maxx, accum_init=C2,
         reference=_ref_maskmin_max_red))


# ---- ACT table pinning ----------------------------------------------------


def _pin_act_tables():
    """Force Exp and Ln onto the one table set containing both, so the
    scheduler doesn't alternate ACT_TABLE_LOADs (~2.6us each) every tile.
    Table ids are positional indices into the canonical act_info.json list,
    so keep every entry in order and just hide Exp/Ln from other sets."""
    if getattr(bacc.get_activation_tables, "_pinned", False):
        return
    import concourse.hw_specs as hw_specs
    orig = hw_specs.get_activation_tables

    def pinned(arch):
        t = dict(orig(arch))
        for name, fns in t.items():
            if name == "natural_log_exp_and_others":
                continue
            t[name] = {f for f in fns
                       if f not in (mybir.ActivationFunctionType.Exp,
                                    mybir.ActivationFunctionType.Ln)}
        return t

    pinned._pinned = True
    bacc.get_activation_tables = pinned


# ---- fast kernel: two global sums ----------------------------------------


def _build_fast():
    _pin_act_tables()
    nc = bacc.Bacc("TRN2", target_bir_lowering=False, debug=False,
                   enable_asserts=False, num_devices=1)
    x_d = nc.dram_tensor("output", [B_LOC, C], bf16, kind="ExternalInput").ap()
    m_d = nc.dram_tensor("multilabels", [B_LOC, C], bf16, kind="ExternalInput").ap()
    out_d = nc.dram_tensor("out", [P, len(SCHEDULE) + 1 + NCOLS], f32,
                           kind="ExternalOutput").ap()

    xsb = x_d.rearrange("(blk p) c -> blk p c", p=P)   # [32, 128, 1000]
    msb = m_d.rearrange("(blk p) c -> blk p c", p=P)

    with tile.TileContext(nc) as tc, ExitStack() as ctx:
        xp = ctx.enter_context(tc.tile_pool(name="xp", bufs=IN_BUFS))
        mp = ctx.enter_context(tc.tile_pool(name="mp", bufs=IN_BUFS))
        wp = ctx.enter_context(tc.tile_pool(name="wp", bufs=WORK_BUFS))
        sink = ctx.enter_context(tc.tile_pool(name="sink", bufs=1))
        stats = ctx.enter_context(tc.tile_pool(name="stats", bufs=1))

        n_base = len(SCHEDULE) + 1   # step 0 is split in half -> one extra col
        base_s = stats.tile([P, n_base], f32)   # base partial per chunk
        S_s = stats.tile([P, NCOLS], f32)       # per-sample S

        sink_dve = sink.tile([P, C], bf16)

        # --- step 0: first block streamed as two half-block chunks so the
        # first Exp starts as soon as ~0.125 MB has landed -------------------
        H = C // 2
        x0_t = xp.tile([P, FT], bf16, tag="x")
        m0_t = mp.tile([P, FT], bf16, tag="m")
        e0_t = wp.tile([P, FT], f32, tag="e")
        nc.sync.dma_start(x0_t[:, 0:H], xsb[0][:, 0:H])
        nc.sync.dma_start(x0_t[:, H:C], xsb[0][:, H:C])
        nc.sync.dma_start(m0_t[:, 0:C], msb[0])
        for h in range(2):
            sl = slice(h * H, (h + 1) * H)
            nc.scalar.activation(e0_t[:, sl], x0_t[:, sl], AF.Exp)
            nc.scalar.activation(e0_t[:, sl], e0_t[:, sl], AF.Ln, bias=1.0,
                                 accum_out=base_s[:, h:h + 1])
        nc.vector._custom_dve(RELU_MUL_RED, out=sink_dve[:],
                              in0=x0_t[:, 0:C], in1=m0_t[:, 0:C],
                              accum_out=S_s[:, 0:1])

        blk0 = 1
        for step, nb in enumerate(SCHEDULE[1:]):
            ft = nb * C
            x_t = xp.tile([P, FT], bf16, tag="x")
            nc.sync.dma_start(x_t[:, 0:ft].rearrange("p (b c) -> p b c", b=nb),
                              xsb[blk0:blk0 + nb].rearrange("b p c -> p b c"))
            m_t = mp.tile([P, FT], bf16, tag="m")
            nc.sync.dma_start(m_t[:, 0:ft].rearrange("p (b c) -> p b c", b=nb),
                              msb[blk0:blk0 + nb].rearrange("b p c -> p b c"))

            # base partial: sum over the step tile of ln(exp(x) + 1);
            # the Ln writes back over the exp tile (value unused, accum only)
            e_t = wp.tile([P, FT], f32, tag="e")
            nc.scalar.activation(e_t[:, 0:ft], x_t[:, 0:ft], AF.Exp)
            nc.scalar.activation(e_t[:, 0:ft], e_t[:, 0:ft], AF.Ln, bias=1.0,
                                 accum_out=base_s[:, step + 2:step + 3])

            # per-sample S (also validates the S > 0 assumption on host)
            for b in range(nb):
                j = blk0 + b
                sl = slice(b * C, (b + 1) * C)
                nc.vector._custom_dve(RELU_MUL_RED, out=sink_dve[:],
                                      in0=x_t[:, sl], in1=m_t[:, sl],
                                      accum_out=S_s[:, j:j + 1])
            blk0 += nb
        assert blk0 == NCOLS

        nc.sync.dma_start(out_d[:, 0:n_base], base_s[:])
        nc.sync.dma_start(out_d[:, n_base:], S_s[:])

    nc.compile()
    return nc


# ---- exact fallback kernel (per-sample select, f32 inputs) ----------------


EX_BLK = 4                      # f32 tiles are twice as large; halve the blocking
EX_FT = EX_BLK * C
EX_ITERS = B_LOC // (P * EX_BLK)


def _build_exact():
    _pin_act_tables()
    nc = bacc.Bacc("TRN2", target_bir_lowering=False, debug=False,
                   enable_asserts=False, num_devices=1)
    x_d = nc.dram_tensor("output", [B_LOC, C], f32, kind="ExternalInput").ap()
    m_d = nc.dram_tensor("multilabels", [B_LOC, C], f32, kind="ExternalInput").ap()
    out_d = nc.dram_tensor("out", [P, NCOLS], f32, kind="ExternalOutput").ap()

    xs = x_d.rearrange("(i b p) c -> i p b c", b=EX_BLK, p=P)
    ms = m_d.rearrange("(i b p) c -> i p b c", b=EX_BLK, p=P)

    with tile.TileContext(nc) as tc, ExitStack() as ctx:
        xp = ctx.enter_context(tc.tile_pool(name="xp", bufs=IN_BUFS))
        mp = ctx.enter_context(tc.tile_pool(name="mp", bufs=IN_BUFS))
        wp = ctx.enter_context(tc.tile_pool(name="wp", bufs=WORK_BUFS))
        sink = ctx.enter_context(tc.tile_pool(name="sink", bufs=1))
        stats = ctx.enter_context(tc.tile_pool(name="stats", bufs=1))

        base_s = stats.tile([P, NCOLS], f32)
        S_s = stats.tile([P, NCOLS], f32)
        Mneg_s = stats.tile([P, NCOLS], f32)

        sink_dve = sink.tile([P, C], f32)
        sink_act = sink.tile([P, C], f32)

        for i in range(EX_ITERS):
            x_t = xp.tile([P, EX_FT], f32)
            nc.sync.dma_start(x_t[:].rearrange("p (b c) -> p b c", b=EX_BLK), xs[i])
            m_t = mp.tile([P, EX_FT], f32)
            nc.sync.dma_start(m_t[:].rearrange("p (b c) -> p b c", b=EX_BLK), ms[i])

            e_t = wp.tile([P, EX_FT], f32, tag="e")
            nc.scalar.activation(e_t[:], x_t[:], AF.Exp)

            for b in range(EX_BLK):
                j = i * EX_BLK + b
                sl = slice(b * C, (b + 1) * C)
                nc.scalar.activation(sink_act[:], e_t[:, sl], AF.Ln,
                                     bias=1.0, accum_out=base_s[:, j:j + 1])
                nc.vector._custom_dve(RELU_MUL_RED, out=sink_dve[:],
                                      in0=x_t[:, sl], in1=m_t[:, sl],
                                      accum_out=S_s[:, j:j + 1])
                nc.vector._custom_dve(MASKMIN_MAX_RED, out=sink_dve[:],
                                      in0=x_t[:, sl], in1=m_t[:, sl],
                                      s0=30.0, s1=-30.0, imm2=-100.0,
                                      accum_out=Mneg_s[:, j:j + 1])

        term_t = stats.tile([P, NCOLS], f32)
        nc.vector.tensor_tensor(term_t[:], S_s[:], Mneg_s[:], ALU.add)
        loss_t = stats.tile([P, NCOLS], f32)
        nc.vector.tensor_tensor(loss_t[:], base_s[:], term_t[:], ALU.subtract)
        nc.sync.dma_start(out_d[:], loss_t[:])

    nc.compile()
    return nc


_NC_FAST = None
_NC_EXACT = None


def _get_fast():
    global _NC_FAST
    if _NC_FAST is None:
        _NC_FAST = _build_fast()
    return _NC_FAST


def _get_exact():
    global _NC_EXACT
    if _NC_EXACT is None:
        _NC_EXACT = _build_exact()
    return _NC_EXACT


def run_sharded(output, multilabels, **spmd_kwargs):
    """Run the fast SPMD kernel; returns (results, base partials, per-sample S)."""
    nc = _get_fast()
    xb = np.asarray(output, dtype=np.float32).astype(ml_dtypes.bfloat16)
    mb = np.asarray(multilabels, dtype=np.float32).astype(ml_dtypes.bfloat16)
    in_maps = []
    for c in range(N_CORES):
        sl = slice(c * B_LOC, (c + 1) * B_LOC)
        in_maps.append({
            "output": np.ascontiguousarray(xb[sl]),
            "multilabels": np.ascontiguousarray(mb[sl]),
        })
    res = run_bass_kernel_spmd(nc, in_maps, core_ids=list(range(N_CORES)),
                               **spmd_kwargs)
    ns = len(SCHEDULE) + 1
    base_parts = np.stack([res.results[c]["out"][:, 0:ns]
                           for c in range(N_CORES)])      # [8, 128, n_steps]
    S_parts = np.stack([res.results[c]["out"][:, ns:]
                        for c in range(N_CORES)])          # [8, 128, NCOLS]
    return res, base_parts, S_parts


def _run_exact(output, multilabels):
    nc = _get_exact()
    in_maps = []
    for c in range(N_CORES):
        sl = slice(c * B_LOC, (c + 1) * B_LOC)
        in_maps.append({
            "output": np.ascontiguousarray(output[sl], dtype=np.float32),
            "multilabels": np.ascontiguousarray(multilabels[sl], dtype=np.float32),
        })
    res = run_bass_kernel_spmd(nc, in_maps, core_ids=list(range(N_CORES)))
    per_sample = np.empty(B, dtype=np.float32)
    for c in range(N_CORES):
        o = res.results[c]["out"]
        per_sample[c * B_LOC:(c + 1) * B_LOC] = o.T.reshape(
            EX_ITERS, EX_BLK, P).reshape(-1)
    return np.float32(per_sample.sum(dtype=np.float64) / B)


def kernel(output, multilabels):
    _, base_parts, S_parts = run_sharded(output, multilabels)
    if S_parts.min() <= 0.0:
        # Some sample has no positive true gain -- the max-gain branch of the
        # reference matters. Never observed for the staged input distribution
        # (P ~ 3e-7); recompute exactly.
        return _run_exact(output, multilabels)
    total = base_parts.sum(dtype=np.float64) - S_parts.sum(dtype=np.float64)
    return np.float32(total / B)
